# revision 2
# baseline (speedup 1.0000x reference)
"""Scatter-max of E edges into an [n, n] f32 matrix on 8 TRN2 NeuronCores.

Strategy (1D row sharding):
  - Host: route edges to cores by row block (1024 rows/core), dedup duplicate
    (row, col) cells keeping the max weight (sort by cell key with weight
    tiebreak), and pack each edge as two u16 halves (f32 bit halves) with
    in-chunk u16 indices, bucketed by (rowgroup, colchunk, partition).
  - Device (per core): for each of 128 chunks (8 rowgroups x 16 colchunks of
    512 f32 cols), GPSIMD `local_scatter` builds the dense [128, 1024]-u16
    tile (zeros + scattered edge halves) in SBUF; HWDGE DMA writes it to the
    [1024, 16384]-u16 (= [1024, 8192] f32) output block.
  - Host: stack the 8 row blocks.
"""

import os
import sys

for _p in ("/opt/trn_rl_repo", "/root/.axon_site/_ro/trn_rl_repo"):
    if os.path.isdir(_p) and _p not in sys.path:
        sys.path.insert(0, _p)
        break

import numpy as np

N = 8192
NCORES = 8
ROWS_PER_CORE = N // NCORES  # 1024
RG = 8  # rowgroups per core (128 rows each)
CC = 16  # colchunks per core (512 f32 cols each)
NCHUNK = RG * CC  # 128
COLS_PER_CHUNK = N // CC  # 512 f32 cols
NELEMS = 2 * COLS_PER_CHUNK  # 1024 u16 per partition per chunk
P = 128

_kernel_cache = {}


def _build_bass_kernel(ni: int):
    import concourse.tile as tile
    from concourse import bacc, mybir

    nc = bacc.Bacc("TRN2", debug=False, num_devices=NCORES)
    idxs_d = nc.dram_tensor(
        "idxs", [NCHUNK, P, ni], mybir.dt.int16, kind="ExternalInput"
    ).ap()
    data_d = nc.dram_tensor(
        "data", [NCHUNK, P, ni], mybir.dt.uint16, kind="ExternalInput"
    ).ap()
    out_d = nc.dram_tensor(
        "out", [ROWS_PER_CORE, 2 * N], mybir.dt.uint16, kind="ExternalOutput"
    ).ap()

    with tile.TileContext(nc) as tc:
        with (
            tc.tile_pool(name="io", bufs=4) as iop,
            tc.tile_pool(name="dense", bufs=4) as dp,
        ):
            for ch in range(NCHUNK):
                g, j = divmod(ch, CC)
                it = iop.tile([P, ni], mybir.dt.int16)
                dt_ = iop.tile([P, ni], mybir.dt.uint16)
                nc.sync.dma_start(out=it[:], in_=idxs_d[ch])
                nc.sync.dma_start(out=dt_[:], in_=data_d[ch])
                dn = dp.tile([P, NELEMS], mybir.dt.uint16)
                nc.gpsimd.local_scatter(
                    out_ap=dn[:],
                    data_ap=dt_[:],
                    idxs_ap=it[:],
                    channels=P,
                    num_elems=NELEMS,
                    num_idxs=ni,
                )
                nc.scalar.dma_start(
                    out=out_d[
                        g * P : (g + 1) * P,
                        j * NELEMS : (j + 1) * NELEMS,
                    ],
                    in_=dn[:],
                )
    nc.compile()
    return nc


def _prepare_inputs(weights, rows, cols):
    """Route + dedup + pack edges. Returns (idxs_all, data_all, ni):
    idxs_all [NCORES, NCHUNK, P, ni] i16, data_all same shape u16."""
    r = np.ascontiguousarray(np.asarray(rows)).astype(np.int64, copy=False)
    c = np.ascontiguousarray(np.asarray(cols)).astype(np.int64, copy=False)
    w = np.ascontiguousarray(np.asarray(weights, dtype=np.float32)).view(np.uint32)

    # cell key ordered (core, g, j, p, cloc): bijection of (row, col)
    core = r >> 10
    g = (r >> 7) & 7
    p = r & 127
    j = c >> 9
    cloc = c & 511
    k2 = ((((((core << 3) | g) << 4) | j) << 7) | p) << 9 | cloc

    order = np.lexsort((w, k2))  # by cell, then weight ascending
    k2s = k2[order]
    keep = np.empty(k2s.size, dtype=bool)
    keep[:-1] = k2s[:-1] != k2s[1:]
    keep[-1] = True
    sel = order[keep]  # unique cells, max weight (uniform [0,1) floats: u32
    k2u = k2s[keep]  # order == f32 order for non-negative values)

    grp = k2u >> 9  # (core, g, j, p) group id, contiguous range
    ngroups = NCORES * NCHUNK * P
    starts = np.flatnonzero(np.r_[True, grp[1:] != grp[:-1]])
    counts = np.diff(np.r_[starts, grp.size])
    rank = np.arange(grp.size, dtype=np.int64) - np.repeat(starts, counts)

    ni = int(2 * counts.max())
    ni = (ni + 7) & ~7  # pad to multiple of 8

    idxs_flat = np.full(ngroups * ni, -1, dtype=np.int16)
    data_flat = np.zeros(ngroups * ni, dtype=np.uint16)
    base = grp * ni + 2 * rank
    cl = k2u & 511
    wsel = w[sel]
    idxs_flat[base] = (2 * cl).astype(np.int16)
    idxs_flat[base + 1] = (2 * cl + 1).astype(np.int16)
    data_flat[base] = (wsel & 0xFFFF).astype(np.uint16)
    data_flat[base + 1] = (wsel >> 16).astype(np.uint16)

    idxs_all = idxs_flat.reshape(NCORES, NCHUNK, P, ni)
    data_all = data_flat.reshape(NCORES, NCHUNK, P, ni)
    return idxs_all, data_all, ni


def kernel(weights=None, rows=None, cols=None, n=None, **_ignored):
    from concourse.bass_utils import run_bass_kernel_spmd

    assert int(n) == N
    idxs_all, data_all, ni = _prepare_inputs(weights, rows, cols)

    if ni not in _kernel_cache:
        _kernel_cache[ni] = _build_bass_kernel(ni)
    nc = _kernel_cache[ni]

    in_maps = [
        {"idxs": idxs_all[cid], "data": data_all[cid]} for cid in range(NCORES)
    ]
    res = run_bass_kernel_spmd(nc, in_maps, core_ids=list(range(NCORES)))
    global _last_res
    _last_res = res

    out = np.empty((N, N), dtype=np.float32)
    for cid in range(NCORES):
        blk = np.ascontiguousarray(res.results[cid]["out"])
        out[cid * ROWS_PER_CORE : (cid + 1) * ROWS_PER_CORE] = blk.view(np.float32)
    return out


# revision 3
# speedup vs baseline: 1.1804x; 1.1804x over previous
"""Scatter-max of E edges into an [n, n] f32 matrix on 8 TRN2 NeuronCores.

Strategy (1D row sharding, dense build):
  - Host: route edges to cores by row block (1024 rows/core), dedup duplicate
    (row, col) cells keeping the max weight (single sort by cell key with
    weight tiebreak), pack each edge as two u16 halves (f32 bit halves) with
    in-chunk u16 indices, bucketed by (rowgroup, colchunk, partition).
  - Device (per core): for each of 128 chunks (8 rowgroups x 16 colchunks of
    512 f32 cols), GPSIMD `local_scatter` builds the dense [128, 1024]-u16
    tile (zeros + scattered edge halves) in SBUF; HWDGE DMA writes dense
    tiles to the [1024, 16384]-u16 (= [1024, 8192] f32) output block.
    Input loads are fused 4 chunks per DMA; output stores 2 chunks per DMA.
  - Host: stack the 8 row blocks.
"""

import os
import sys

for _p in ("/opt/trn_rl_repo", "/root/.axon_site/_ro/trn_rl_repo"):
    if os.path.isdir(_p) and _p not in sys.path:
        sys.path.insert(0, _p)
        break

import numpy as np

N = 8192
NCORES = 8
ROWS_PER_CORE = N // NCORES  # 1024
RG = 8  # rowgroups per core (128 rows each)
CC = 16  # colchunks per core (512 f32 cols each)
NCHUNK = RG * CC  # 128
COLS_PER_CHUNK = N // CC  # 512 f32 cols
NELEMS = 2 * COLS_PER_CHUNK  # 1024 u16 per partition per chunk
P = 128
QUAD = 4  # chunks per fused input DMA
PAIR = 2  # chunks per dense tile / output DMA

_kernel_cache = {}
_last_res = None


def _build_bass_kernel(ni: int):
    import concourse.tile as tile
    from concourse import bacc, mybir

    nc = bacc.Bacc("TRN2", debug=False, num_devices=NCORES)
    fin_d = nc.dram_tensor(
        "fin", [NCHUNK // QUAD, P, 2 * QUAD * ni], mybir.dt.uint16,
        kind="ExternalInput",
    ).ap()
    out_d = nc.dram_tensor(
        "out", [ROWS_PER_CORE, 2 * N], mybir.dt.uint16, kind="ExternalOutput"
    ).ap()

    with tile.TileContext(nc) as tc:
        with (
            tc.tile_pool(name="io", bufs=3) as iop,
            tc.tile_pool(name="dense", bufs=4) as dp,
        ):
            for q in range(NCHUNK // QUAD):
                ft = iop.tile([P, 2 * QUAD * ni], mybir.dt.uint16)
                nc.sync.dma_start(out=ft[:], in_=fin_d[q])
                for h in range(QUAD // PAIR):
                    dn = dp.tile([P, PAIR * NELEMS], mybir.dt.uint16)
                    for m in range(PAIR):
                        t = h * PAIR + m
                        ch = q * QUAD + t
                        nc.gpsimd.local_scatter(
                            out_ap=dn[:, m * NELEMS : (m + 1) * NELEMS],
                            data_ap=ft[:, 2 * ni * t + ni : 2 * ni * (t + 1)],
                            idxs_ap=ft[:, 2 * ni * t : 2 * ni * t + ni].bitcast(
                                mybir.dt.int16
                            ),
                            channels=P,
                            num_elems=NELEMS,
                            num_idxs=ni,
                        )
                    ch0 = q * QUAD + h * PAIR
                    g, j = divmod(ch0, CC)
                    nc.scalar.dma_start(
                        out=out_d[
                            g * P : (g + 1) * P,
                            j * NELEMS : j * NELEMS + PAIR * NELEMS,
                        ],
                        in_=dn[:],
                    )
    nc.compile()
    return nc


def _prepare_inputs(weights, rows, cols):
    """Route + dedup + pack edges into the fused input layout.
    Returns (fin_all [NCORES, NCHUNK//QUAD, P, 2*QUAD*ni] u16, ni)."""
    r = np.ascontiguousarray(np.asarray(rows)).astype(np.int64, copy=False)
    c = np.ascontiguousarray(np.asarray(cols)).astype(np.int64, copy=False)
    w = np.ascontiguousarray(np.asarray(weights, dtype=np.float32)).view(np.uint32)

    # cell key ordered (core, g, j, p, cloc): bijection of (row, col)
    core = r >> 10
    g = (r >> 7) & 7
    p = r & 127
    j = c >> 9
    cloc = c & 511
    k2 = ((((((core << 3) | g) << 4) | j) << 7) | p) << 9 | cloc

    order = np.lexsort((w, k2))  # by cell, then weight ascending
    k2s = k2[order]
    keep = np.empty(k2s.size, dtype=bool)
    keep[:-1] = k2s[:-1] != k2s[1:]
    keep[-1] = True
    sel = order[keep]  # unique cells, max weight (uniform [0,1) floats: u32
    k2u = k2s[keep]  # order == f32 order for non-negative values)

    grp = k2u >> 9  # (core, g, j, p) group id, contiguous range
    ngroups = NCORES * NCHUNK * P
    starts = np.flatnonzero(np.r_[True, grp[1:] != grp[:-1]])
    counts = np.diff(np.r_[starts, grp.size])
    rank = np.arange(grp.size, dtype=np.int64) - np.repeat(starts, counts)

    ni = int(2 * counts.max())
    ni = (ni + 7) & ~7  # pad to multiple of 8

    idxs_flat = np.full(ngroups * ni, -1, dtype=np.int16)
    data_flat = np.zeros(ngroups * ni, dtype=np.uint16)
    base = grp * ni + 2 * rank
    cl = k2u & 511
    wsel = w[sel]
    idxs_flat[base] = (2 * cl).astype(np.int16)
    idxs_flat[base + 1] = (2 * cl + 1).astype(np.int16)
    data_flat[base] = (wsel & 0xFFFF).astype(np.uint16)
    data_flat[base + 1] = (wsel >> 16).astype(np.uint16)

    idxs_all = idxs_flat.reshape(NCORES, NCHUNK, P, ni)
    data_all = data_flat.reshape(NCORES, NCHUNK, P, ni)
    # fuse: per chunk [idxs | data] then group QUAD chunks per DMA row
    fused = np.concatenate([idxs_all.view(np.uint16), data_all], axis=-1)
    fin_all = np.ascontiguousarray(
        fused.reshape(NCORES, NCHUNK // QUAD, QUAD, P, 2 * ni)
        .transpose(0, 1, 3, 2, 4)
        .reshape(NCORES, NCHUNK // QUAD, P, 2 * QUAD * ni)
    )
    return fin_all, ni


def kernel(weights=None, rows=None, cols=None, n=None, **_ignored):
    from concourse.bass_utils import run_bass_kernel_spmd

    assert int(n) == N
    fin_all, ni = _prepare_inputs(weights, rows, cols)

    if ni not in _kernel_cache:
        _kernel_cache[ni] = _build_bass_kernel(ni)
    nc = _kernel_cache[ni]

    in_maps = [{"fin": fin_all[cid]} for cid in range(NCORES)]
    res = run_bass_kernel_spmd(nc, in_maps, core_ids=list(range(NCORES)))
    global _last_res
    _last_res = res

    out = np.empty((N, N), dtype=np.float32)
    for cid in range(NCORES):
        blk = np.ascontiguousarray(res.results[cid]["out"])
        out[cid * ROWS_PER_CORE : (cid + 1) * ROWS_PER_CORE] = blk.view(np.float32)
    return out


# revision 4
# speedup vs baseline: 1.1873x; 1.0059x over previous
"""Scatter-max of E edges into an [n, n] f32 matrix on 8 TRN2 NeuronCores.

Strategy (1D row sharding, dense build):
  - Host: route edges to cores by row block (1024 rows/core), dedup duplicate
    (row, col) cells keeping the max weight (single sort by cell key with
    weight tiebreak), pack each edge as two u16 halves (f32 bit halves) with
    in-chunk u16 indices, bucketed by (rowgroup, colchunk, partition).
  - Device (per core): for each of 128 chunks (8 rowgroups x 16 colchunks of
    512 f32 cols), GPSIMD `local_scatter` builds the dense [128, 1024]-u16
    tile (zeros + scattered edge halves) in SBUF; HWDGE DMA writes dense
    tiles to the [1024, 16384]-u16 (= [1024, 8192] f32) output block.
    Input loads are fused 4 chunks per DMA; output stores 2 chunks per DMA.
  - Host: stack the 8 row blocks.
"""

import os
import sys

for _p in ("/opt/trn_rl_repo", "/root/.axon_site/_ro/trn_rl_repo"):
    if os.path.isdir(_p) and _p not in sys.path:
        sys.path.insert(0, _p)
        break

import numpy as np

N = 8192
NCORES = 8
ROWS_PER_CORE = N // NCORES  # 1024
RG = 8  # rowgroups per core (128 rows each)
CC = 16  # colchunks per core (512 f32 cols each)
NCHUNK = RG * CC  # 128
COLS_PER_CHUNK = N // CC  # 512 f32 cols
NELEMS = 2 * COLS_PER_CHUNK  # 1024 u16 per partition per chunk
P = 128
QUAD = 4  # chunks per fused input DMA
PAIR = 2  # chunks per dense tile / output DMA

_kernel_cache = {}
_last_res = None


def _build_bass_kernel(ni: int):
    import concourse.tile as tile
    from concourse import bacc, mybir

    nc = bacc.Bacc("TRN2", debug=False, num_devices=NCORES)
    fin_d = nc.dram_tensor(
        "fin", [NCHUNK // QUAD, P, 2 * QUAD * ni], mybir.dt.uint16,
        kind="ExternalInput",
    ).ap()
    out_d = nc.dram_tensor(
        "out", [ROWS_PER_CORE, 2 * N], mybir.dt.uint16, kind="ExternalOutput"
    ).ap()

    with tile.TileContext(nc) as tc:
        with (
            tc.tile_pool(name="io", bufs=4) as iop,
            tc.tile_pool(name="dense", bufs=8) as dp,
        ):
            for q in range(NCHUNK // QUAD):
                ft = iop.tile([P, 2 * QUAD * ni], mybir.dt.uint16)
                nc.sync.dma_start(out=ft[:], in_=fin_d[q])
                for h in range(QUAD // PAIR):
                    dn = dp.tile([P, PAIR * NELEMS], mybir.dt.uint16)
                    for m in range(PAIR):
                        t = h * PAIR + m
                        ch = q * QUAD + t
                        nc.gpsimd.local_scatter(
                            out_ap=dn[:, m * NELEMS : (m + 1) * NELEMS],
                            data_ap=ft[:, 2 * ni * t + ni : 2 * ni * (t + 1)],
                            idxs_ap=ft[:, 2 * ni * t : 2 * ni * t + ni].bitcast(
                                mybir.dt.int16
                            ),
                            channels=P,
                            num_elems=NELEMS,
                            num_idxs=ni,
                        )
                    ch0 = q * QUAD + h * PAIR
                    g, j = divmod(ch0, CC)
                    nc.scalar.dma_start(
                        out=out_d[
                            g * P : (g + 1) * P,
                            j * NELEMS : j * NELEMS + PAIR * NELEMS,
                        ],
                        in_=dn[:],
                    )
    nc.compile()
    return nc


def _prepare_inputs(weights, rows, cols):
    """Route + dedup + pack edges into the fused input layout.
    Returns (fin_all [NCORES, NCHUNK//QUAD, P, 2*QUAD*ni] u16, ni)."""
    r = np.ascontiguousarray(np.asarray(rows)).astype(np.int64, copy=False)
    c = np.ascontiguousarray(np.asarray(cols)).astype(np.int64, copy=False)
    w = np.ascontiguousarray(np.asarray(weights, dtype=np.float32)).view(np.uint32)

    # cell key ordered (core, g, j, p, cloc): bijection of (row, col)
    core = r >> 10
    g = (r >> 7) & 7
    p = r & 127
    j = c >> 9
    cloc = c & 511
    k2 = ((((((core << 3) | g) << 4) | j) << 7) | p) << 9 | cloc

    order = np.lexsort((w, k2))  # by cell, then weight ascending
    k2s = k2[order]
    keep = np.empty(k2s.size, dtype=bool)
    keep[:-1] = k2s[:-1] != k2s[1:]
    keep[-1] = True
    sel = order[keep]  # unique cells, max weight (uniform [0,1) floats: u32
    k2u = k2s[keep]  # order == f32 order for non-negative values)

    grp = k2u >> 9  # (core, g, j, p) group id, contiguous range
    ngroups = NCORES * NCHUNK * P
    starts = np.flatnonzero(np.r_[True, grp[1:] != grp[:-1]])
    counts = np.diff(np.r_[starts, grp.size])
    rank = np.arange(grp.size, dtype=np.int64) - np.repeat(starts, counts)

    ni = int(2 * counts.max())
    ni = (ni + 7) & ~7  # pad to multiple of 8

    idxs_flat = np.full(ngroups * ni, -1, dtype=np.int16)
    data_flat = np.zeros(ngroups * ni, dtype=np.uint16)
    base = grp * ni + 2 * rank
    cl = k2u & 511
    wsel = w[sel]
    idxs_flat[base] = (2 * cl).astype(np.int16)
    idxs_flat[base + 1] = (2 * cl + 1).astype(np.int16)
    data_flat[base] = (wsel & 0xFFFF).astype(np.uint16)
    data_flat[base + 1] = (wsel >> 16).astype(np.uint16)

    idxs_all = idxs_flat.reshape(NCORES, NCHUNK, P, ni)
    data_all = data_flat.reshape(NCORES, NCHUNK, P, ni)
    # fuse: per chunk [idxs | data] then group QUAD chunks per DMA row
    fused = np.concatenate([idxs_all.view(np.uint16), data_all], axis=-1)
    fin_all = np.ascontiguousarray(
        fused.reshape(NCORES, NCHUNK // QUAD, QUAD, P, 2 * ni)
        .transpose(0, 1, 3, 2, 4)
        .reshape(NCORES, NCHUNK // QUAD, P, 2 * QUAD * ni)
    )
    return fin_all, ni


def kernel(weights=None, rows=None, cols=None, n=None, **_ignored):
    from concourse.bass_utils import run_bass_kernel_spmd

    assert int(n) == N
    fin_all, ni = _prepare_inputs(weights, rows, cols)

    if ni not in _kernel_cache:
        _kernel_cache[ni] = _build_bass_kernel(ni)
    nc = _kernel_cache[ni]

    in_maps = [{"fin": fin_all[cid]} for cid in range(NCORES)]
    res = run_bass_kernel_spmd(nc, in_maps, core_ids=list(range(NCORES)))
    global _last_res
    _last_res = res

    out = np.empty((N, N), dtype=np.float32)
    for cid in range(NCORES):
        blk = np.ascontiguousarray(res.results[cid]["out"])
        out[cid * ROWS_PER_CORE : (cid + 1) * ROWS_PER_CORE] = blk.view(np.float32)
    return out


# revision 5
# speedup vs baseline: 1.2859x; 1.0831x over previous
"""Scatter-max of E edges into an [n, n] f32 matrix on 8 TRN2 NeuronCores.

Strategy (1D row sharding, dense build):
  - Host: route edges to cores by row block (1024 rows/core), dedup duplicate
    (row, col) cells keeping the max weight (single sort by cell key with
    weight tiebreak), pack each edge as two u16 halves (f32 bit halves) with
    in-chunk u16 indices, bucketed by (rowgroup, colchunk, partition).
  - Device (per core): per rowgroup (128 rows), 8 wide colchunks of 1023 f32
    cols (2046 u16 = GPSIMD local_scatter num_elems limit) plus one 8-col
    tail chunk. GPSIMD `local_scatter` builds each dense chunk (zeros +
    scattered edge halves) in SBUF; HWDGE DMA writes pairs of chunks to the
    [1024, 16384]-u16 (= [1024, 8192] f32) output block.
  - Host: stack the 8 row blocks.
"""

import os
import sys

for _p in ("/opt/trn_rl_repo", "/root/.axon_site/_ro/trn_rl_repo"):
    if os.path.isdir(_p) and _p not in sys.path:
        sys.path.insert(0, _p)
        break

import numpy as np

N = 8192
NCORES = 8
ROWS_PER_CORE = N // NCORES  # 1024
RG = 8  # rowgroups per core (128 rows each)
P = 128
WBIG = 1023  # f32 cols per big chunk (2*WBIG = 2046 <= ucode num_elems limit)
NBIG = 8  # big chunks per rowgroup
WTAIL = N - NBIG * WBIG  # 8 f32 cols
NE_B = 2 * WBIG  # 2046
NE_T = 2 * WTAIL  # 16

_kernel_cache = {}
_last_res = None


def _build_bass_kernel(nb: int, nt: int):
    import concourse.tile as tile
    from concourse import bacc, mybir

    ln = NBIG * 2 * nb + 2 * nt  # u16 per partition per rowgroup input row
    nc = bacc.Bacc("TRN2", debug=False, num_devices=NCORES)
    fin_d = nc.dram_tensor(
        "fin", [RG, P, ln], mybir.dt.uint16, kind="ExternalInput"
    ).ap()
    out_d = nc.dram_tensor(
        "out", [ROWS_PER_CORE, 2 * N], mybir.dt.uint16, kind="ExternalOutput"
    ).ap()

    with tile.TileContext(nc) as tc:
        with (
            tc.tile_pool(name="io", bufs=3) as iop,
            tc.tile_pool(name="dense", bufs=5) as dp,
            tc.tile_pool(name="tail", bufs=2) as tp,
        ):
            for g in range(RG):
                ft = iop.tile([P, ln], mybir.dt.uint16)
                nc.sync.dma_start(out=ft[:], in_=fin_d[g])
                rows = slice(g * P, (g + 1) * P)
                for h in range(NBIG // 2):
                    dn = dp.tile([P, 2 * NE_B], mybir.dt.uint16)
                    for m in range(2):
                        j = 2 * h + m
                        off = j * 2 * nb
                        nc.gpsimd.local_scatter(
                            out_ap=dn[:, m * NE_B : (m + 1) * NE_B],
                            data_ap=ft[:, off + nb : off + 2 * nb],
                            idxs_ap=ft[:, off : off + nb].bitcast(mybir.dt.int16),
                            channels=P,
                            num_elems=NE_B,
                            num_idxs=nb,
                        )
                    c0 = 2 * h * NE_B
                    nc.scalar.dma_start(
                        out=out_d[rows, c0 : c0 + 2 * NE_B], in_=dn[:]
                    )
                offt = NBIG * 2 * nb
                dnt = tp.tile([P, NE_T], mybir.dt.uint16)
                nc.gpsimd.local_scatter(
                    out_ap=dnt[:],
                    data_ap=ft[:, offt + nt : offt + 2 * nt],
                    idxs_ap=ft[:, offt : offt + nt].bitcast(mybir.dt.int16),
                    channels=P,
                    num_elems=NE_T,
                    num_idxs=nt,
                )
                nc.scalar.dma_start(
                    out=out_d[rows, NBIG * NE_B : NBIG * NE_B + NE_T], in_=dnt[:]
                )
    nc.compile()
    return nc


def _prepare_inputs(weights, rows, cols):
    """Route + dedup + pack edges into the fused per-rowgroup input layout.
    Returns (fin_all [NCORES, RG, P, ln] u16, nb, nt)."""
    r = np.ascontiguousarray(np.asarray(rows)).astype(np.int64, copy=False)
    c = np.ascontiguousarray(np.asarray(cols)).astype(np.int64, copy=False)
    w = np.ascontiguousarray(np.asarray(weights, dtype=np.float32)).view(np.uint32)

    core = r >> 10
    g = (r >> 7) & 7
    p = r & 127
    j = c // WBIG  # 0..8 (j == 8 is the tail chunk)
    cloc = c - j * WBIG
    # cell key ordered (core, g, j, p, cloc): bijection of (row, col)
    k2 = ((((((core << 3) | g) << 4) | j) << 7) | p) << 10 | cloc

    order = np.lexsort((w, k2))  # by cell, then weight ascending
    k2s = k2[order]
    keep = np.empty(k2s.size, dtype=bool)
    keep[:-1] = k2s[:-1] != k2s[1:]
    keep[-1] = True
    sel = order[keep]  # unique cells, max weight (uniform [0,1) floats: u32
    k2u = k2s[keep]  # order == f32 order for non-negative values)

    grp = k2u >> 10  # (core, g, j, p) group id
    jj = (grp >> 7) & 15
    starts = np.flatnonzero(np.r_[True, grp[1:] != grp[:-1]])
    counts = np.diff(np.r_[starts, grp.size])
    rank = np.arange(grp.size, dtype=np.int64) - np.repeat(starts, counts)

    cnt_big = counts[jj[starts] < NBIG].max() if np.any(jj[starts] < NBIG) else 1
    cnt_tail = counts[jj[starts] == NBIG].max() if np.any(jj[starts] == NBIG) else 1
    nb = (int(2 * cnt_big) + 7) & ~7
    nt = max(16, (int(2 * cnt_tail) + 7) & ~7)
    ln = NBIG * 2 * nb + 2 * nt

    # flat position of each edge's two u16 entries
    corege = grp >> 11  # (core, g)
    pp = grp & 127
    off_j = np.where(jj < NBIG, jj * 2 * nb, NBIG * 2 * nb)
    ni_j = np.where(jj < NBIG, nb, nt)
    row_base = (corege * P + pp) * ln
    idx_pos = row_base + off_j + 2 * rank
    dat_pos = idx_pos + ni_j

    fin = np.zeros(NCORES * RG * P * ln, dtype=np.uint16)
    # initialize all idx slots to -1
    iview = fin.view(np.int16)
    for jv in range(NBIG + 1):
        o = jv * 2 * nb if jv < NBIG else NBIG * 2 * nb
        w_j = nb if jv < NBIG else nt
        sl = iview.reshape(NCORES * RG * P, ln)[:, o : o + w_j]
        sl[:] = -1
    cl = k2u & 1023
    wsel = w[sel]
    iview[idx_pos] = (2 * cl).astype(np.int16)
    iview[idx_pos + 1] = (2 * cl + 1).astype(np.int16)
    fin[dat_pos] = (wsel & 0xFFFF).astype(np.uint16)
    fin[dat_pos + 1] = (wsel >> 16).astype(np.uint16)

    return fin.reshape(NCORES, RG, P, ln), nb, nt


def kernel(weights=None, rows=None, cols=None, n=None, **_ignored):
    from concourse.bass_utils import run_bass_kernel_spmd

    assert int(n) == N
    fin_all, nb, nt = _prepare_inputs(weights, rows, cols)

    key = (nb, nt)
    if key not in _kernel_cache:
        _kernel_cache[key] = _build_bass_kernel(nb, nt)
    nc = _kernel_cache[key]

    in_maps = [{"fin": fin_all[cid]} for cid in range(NCORES)]
    res = run_bass_kernel_spmd(nc, in_maps, core_ids=list(range(NCORES)))
    global _last_res
    _last_res = res

    out = np.empty((N, N), dtype=np.float32)
    for cid in range(NCORES):
        blk = np.ascontiguousarray(res.results[cid]["out"])
        out[cid * ROWS_PER_CORE : (cid + 1) * ROWS_PER_CORE] = blk.view(np.float32)
    return out


# revision 6
# speedup vs baseline: 1.3057x; 1.0154x over previous
"""Scatter-max of E edges into an [n, n] f32 matrix on 8 TRN2 NeuronCores.

Strategy (1D row sharding, dense build):
  - Host: route edges to cores by row block (1024 rows/core), dedup duplicate
    (row, col) cells keeping the max weight (single sort by cell key with
    weight tiebreak), pack each edge as two u16 halves (f32 bit halves) with
    in-chunk u16 indices, bucketed by (rowgroup, colchunk, partition).
  - Device (per core): per rowgroup (128 rows), 8 wide colchunks of 1023 f32
    cols (2046 u16 = GPSIMD local_scatter num_elems limit) plus one 8-col
    tail chunk. GPSIMD `local_scatter` builds each dense chunk (zeros +
    scattered edge halves) in SBUF; HWDGE DMA writes pairs of chunks to the
    [1024, 16384]-u16 (= [1024, 8192] f32) output block.
  - Host: stack the 8 row blocks.
"""

import os
import sys

for _p in ("/opt/trn_rl_repo", "/root/.axon_site/_ro/trn_rl_repo"):
    if os.path.isdir(_p) and _p not in sys.path:
        sys.path.insert(0, _p)
        break

import numpy as np

N = 8192
NCORES = 8
ROWS_PER_CORE = N // NCORES  # 1024
RG = 8  # rowgroups per core (128 rows each)
P = 128
WBIG = 1023  # f32 cols per big chunk (2*WBIG = 2046 <= ucode num_elems limit)
NBIG = 8  # big chunks per rowgroup
WTAIL = N - NBIG * WBIG  # 8 f32 cols
NE_B = 2 * WBIG  # 2046
NE_T = 2 * WTAIL  # 16

_kernel_cache = {}
_last_res = None


def _build_bass_kernel(nb: int, nt: int):
    import concourse.tile as tile
    from concourse import bacc, mybir

    ln = NBIG * 2 * nb + 2 * nt  # u16 per partition per rowgroup input row
    nc = bacc.Bacc("TRN2", debug=False, num_devices=NCORES)
    fin_d = nc.dram_tensor(
        "fin", [RG, P, ln], mybir.dt.uint16, kind="ExternalInput"
    ).ap()
    out_d = nc.dram_tensor(
        "out", [ROWS_PER_CORE, 2 * N], mybir.dt.uint16, kind="ExternalOutput"
    ).ap()

    with tile.TileContext(nc) as tc:
        with (
            tc.tile_pool(name="io", bufs=4) as iop,
            tc.tile_pool(name="dense", bufs=8) as dp,
            tc.tile_pool(name="tail", bufs=2) as tp,
        ):
            half = 2 * 2 * nb  # u16 offset after first 2 big chunks
            for g in range(RG):
                ft = iop.tile([P, ln], mybir.dt.uint16)
                nc.sync.dma_start(out=ft[:, :half], in_=fin_d[g][:, :half])
                nc.sync.dma_start(out=ft[:, half:], in_=fin_d[g][:, half:])
                rows = slice(g * P, (g + 1) * P)
                for h in range(NBIG // 2):
                    dn = dp.tile([P, 2 * NE_B], mybir.dt.uint16)
                    for m in range(2):
                        j = 2 * h + m
                        off = j * 2 * nb
                        nc.gpsimd.local_scatter(
                            out_ap=dn[:, m * NE_B : (m + 1) * NE_B],
                            data_ap=ft[:, off + nb : off + 2 * nb],
                            idxs_ap=ft[:, off : off + nb].bitcast(mybir.dt.int16),
                            channels=P,
                            num_elems=NE_B,
                            num_idxs=nb,
                        )
                    c0 = 2 * h * NE_B
                    out_eng = nc.scalar if h % 2 == 0 else nc.sync
                    out_eng.dma_start(
                        out=out_d[rows, c0 : c0 + 2 * NE_B], in_=dn[:]
                    )
                offt = NBIG * 2 * nb
                dnt = tp.tile([P, NE_T], mybir.dt.uint16)
                nc.gpsimd.local_scatter(
                    out_ap=dnt[:],
                    data_ap=ft[:, offt + nt : offt + 2 * nt],
                    idxs_ap=ft[:, offt : offt + nt].bitcast(mybir.dt.int16),
                    channels=P,
                    num_elems=NE_T,
                    num_idxs=nt,
                )
                nc.scalar.dma_start(
                    out=out_d[rows, NBIG * NE_B : NBIG * NE_B + NE_T], in_=dnt[:]
                )
    nc.compile()
    return nc


def _prepare_inputs(weights, rows, cols):
    """Route + dedup + pack edges into the fused per-rowgroup input layout.
    Returns (fin_all [NCORES, RG, P, ln] u16, nb, nt)."""
    r = np.ascontiguousarray(np.asarray(rows)).astype(np.int64, copy=False)
    c = np.ascontiguousarray(np.asarray(cols)).astype(np.int64, copy=False)
    w = np.ascontiguousarray(np.asarray(weights, dtype=np.float32)).view(np.uint32)

    core = r >> 10
    g = (r >> 7) & 7
    p = r & 127
    j = c // WBIG  # 0..8 (j == 8 is the tail chunk)
    cloc = c - j * WBIG
    # cell key ordered (core, g, j, p, cloc): bijection of (row, col)
    k2 = ((((((core << 3) | g) << 4) | j) << 7) | p) << 10 | cloc

    order = np.lexsort((w, k2))  # by cell, then weight ascending
    k2s = k2[order]
    keep = np.empty(k2s.size, dtype=bool)
    keep[:-1] = k2s[:-1] != k2s[1:]
    keep[-1] = True
    sel = order[keep]  # unique cells, max weight (uniform [0,1) floats: u32
    k2u = k2s[keep]  # order == f32 order for non-negative values)

    grp = k2u >> 10  # (core, g, j, p) group id
    jj = (grp >> 7) & 15
    starts = np.flatnonzero(np.r_[True, grp[1:] != grp[:-1]])
    counts = np.diff(np.r_[starts, grp.size])
    rank = np.arange(grp.size, dtype=np.int64) - np.repeat(starts, counts)

    cnt_big = counts[jj[starts] < NBIG].max() if np.any(jj[starts] < NBIG) else 1
    cnt_tail = counts[jj[starts] == NBIG].max() if np.any(jj[starts] == NBIG) else 1
    nb = (int(2 * cnt_big) + 7) & ~7
    nt = max(16, (int(2 * cnt_tail) + 7) & ~7)
    ln = NBIG * 2 * nb + 2 * nt

    # flat position of each edge's two u16 entries
    corege = grp >> 11  # (core, g)
    pp = grp & 127
    off_j = np.where(jj < NBIG, jj * 2 * nb, NBIG * 2 * nb)
    ni_j = np.where(jj < NBIG, nb, nt)
    row_base = (corege * P + pp) * ln
    idx_pos = row_base + off_j + 2 * rank
    dat_pos = idx_pos + ni_j

    fin = np.zeros(NCORES * RG * P * ln, dtype=np.uint16)
    # initialize all idx slots to -1
    iview = fin.view(np.int16)
    for jv in range(NBIG + 1):
        o = jv * 2 * nb if jv < NBIG else NBIG * 2 * nb
        w_j = nb if jv < NBIG else nt
        sl = iview.reshape(NCORES * RG * P, ln)[:, o : o + w_j]
        sl[:] = -1
    cl = k2u & 1023
    wsel = w[sel]
    iview[idx_pos] = (2 * cl).astype(np.int16)
    iview[idx_pos + 1] = (2 * cl + 1).astype(np.int16)
    fin[dat_pos] = (wsel & 0xFFFF).astype(np.uint16)
    fin[dat_pos + 1] = (wsel >> 16).astype(np.uint16)

    return fin.reshape(NCORES, RG, P, ln), nb, nt


def kernel(weights=None, rows=None, cols=None, n=None, **_ignored):
    from concourse.bass_utils import run_bass_kernel_spmd

    assert int(n) == N
    fin_all, nb, nt = _prepare_inputs(weights, rows, cols)

    key = (nb, nt)
    if key not in _kernel_cache:
        _kernel_cache[key] = _build_bass_kernel(nb, nt)
    nc = _kernel_cache[key]

    in_maps = [{"fin": fin_all[cid]} for cid in range(NCORES)]
    res = run_bass_kernel_spmd(nc, in_maps, core_ids=list(range(NCORES)))
    global _last_res
    _last_res = res

    out = np.empty((N, N), dtype=np.float32)
    for cid in range(NCORES):
        blk = np.ascontiguousarray(res.results[cid]["out"])
        out[cid * ROWS_PER_CORE : (cid + 1) * ROWS_PER_CORE] = blk.view(np.float32)
    return out
